# revision 13
# baseline (speedup 1.0000x reference)
"""Trainium2 Bass kernel for nn_CAMEncoder3 (2-layer GATv2 GNN encoder).

Self-contained: kernel(**inputs) -> np.ndarray [50000, 192] float32.

v2: node-range sharding over 8 NeuronCores. Edges bucketed per core by
destination 128-node block and lo/hi table half (int16 gather range), padded
to 128-edge subtiles under a schedule shared by all cores (SPMD). Gather
chunks span blocks (CH subtiles per dma_gather call) to amortize Q7
descriptor-generation cost. Per chunk: gather src/dst rows, leaky-relu
attention logits, exp, weighted rows; per block: one-hot matmuls accumulate
values+denominators in PSUM, then one PSUM->SBUF staging copy. Epilogues
(normalize, head-mean, bias, relu) run batched over all 49 blocks per conv.
Layer-1 output feeds layer-2 tables on-device; XL tables built shard-wise
and AllGathered (DRAM).
"""
import os
import sys

sys.path.insert(0, '/opt/trn_rl_repo')

import numpy as np
import ml_dtypes
import concourse.bacc as bacc
import concourse.mybir as mybir
import concourse.tile as tile
from concourse.bass_utils import run_bass_kernel_spmd
from concourse.library_config import mlp

bf16 = mybir.dt.bfloat16
f32 = mybir.dt.float32
i16 = mybir.dt.int16
AT = mybir.AluOpType
AF = mybir.ActivationFunctionType

CH = 8          # subtiles per gather chunk
PADLOC = 200.0  # dst_local sentinel for pad edges


def _wrap16(a):
    """int16 stream -> dma_gather wrapped layout [128, n/16]."""
    n = a.shape[0]
    w = np.zeros((16, (n + 15) // 16), np.int16)
    w[np.arange(n) % 16, np.arange(n) // 16] = a
    return np.tile(w, (8, 1))


def preprocess_conv(src, dst, ncores, npc, npad, split):
    """Bucket edges per core/block/half, pad to subtiles, build shared
    schedule and per-core streams.

    Subtile order: all lo subtiles block-major, then all hi subtiles
    block-major. Returns (sched, ns_lo, ns_hi, percore_streams)."""
    nb = npad // 128
    rows = (src // npc) * npad + (src % npc)  # AG table row of src
    percore = []
    for c in range(ncores):
        sel = (dst >= c * npc) & (dst < (c + 1) * npc)
        r, d = rows[sel], dst[sel] - c * npc
        blk = d // 128
        lo = r < split
        blocks = []
        for b in range(nb):
            m = blk == b
            blocks.append((r[m & lo], d[m & lo], r[m & ~lo], d[m & ~lo]))
        percore.append(blocks)
    sched = []
    for b in range(nb):
        nlo = max(len(percore[c][b][0]) for c in range(ncores))
        nhi = max(len(percore[c][b][2]) for c in range(ncores))
        nlo = (nlo + 127) // 128
        nhi = (nhi + 127) // 128
        if nlo + nhi == 0:
            nlo = 1
        sched.append((nlo, nhi))
    ns_lo = sum(s[0] for s in sched)
    ns_hi = sum(s[1] for s in sched)
    ns = ns_lo + ns_hi
    out = []
    for c in range(ncores):
        s16 = np.zeros(ns * 128, np.int64)
        d16 = np.zeros(ns * 128, np.int64)
        dl = np.full(ns * 128, PADLOC, np.float32)
        olo, ohi = 0, ns_lo
        for b in range(nb):
            rl, dloc_l, rh, dloc_h = percore[c][b]
            nlo, nhi = sched[b]
            e0 = olo * 128
            s16[e0:e0 + len(rl)] = rl
            d16[e0:e0 + len(rl)] = dloc_l
            dl[e0:e0 + len(rl)] = dloc_l % 128
            olo += nlo
            e0 = ohi * 128
            s16[e0:e0 + len(rh)] = rh - split
            d16[e0:e0 + len(rh)] = dloc_h
            dl[e0:e0 + len(rh)] = dloc_h % 128
            ohi += nhi
        dloc_col = dl.reshape(ns, 128).T.astype(ml_dtypes.bfloat16)
        out.append({
            "src16": _wrap16(s16.astype(np.int16)),
            "dst16": _wrap16(d16.astype(np.int16)),
            "dloc": dloc_col,
        })
    return sched, ns_lo, ns_hi, out


def aug_w(W, b):
    """[din, HC],[HC] -> bf16 [din+1, HC] with bias row appended."""
    return np.vstack([W, b[None, :]]).astype(ml_dtypes.bfloat16)


def build_program(ncores, npc, npad, split, scheds):
    nb = npad // 128
    npadg = ncores * npad
    nc = bacc.Bacc("TRN2", target_bir_lowering=False, debug=False,
                   num_devices=ncores, num_swdge_queues=4)

    D1, D2 = 128, 256   # HC per layer
    xbt = nc.dram_tensor("xbt", [65, npad], bf16, kind="ExternalInput")
    iotin = nc.dram_tensor("iot", [1, 128], bf16, kind="ExternalInput")
    identin = nc.dram_tensor("ident", [128, 128], bf16, kind="ExternalInput")
    convs = {}
    for p, hc in [("j1", D1), ("m1", D1), ("j2", D2), ("m2", D2)]:
        ns = scheds[p][1] + scheds[p][2]
        t = {}
        t["src16"] = nc.dram_tensor(f"{p}_src16", [128, ns * 8], i16,
                                    kind="ExternalInput")
        t["dst16"] = nc.dram_tensor(f"{p}_dst16", [128, ns * 8], i16,
                                    kind="ExternalInput")
        t["dloc"] = nc.dram_tensor(f"{p}_dloc", [128, ns], bf16,
                                   kind="ExternalInput")
        t["att"] = nc.dram_tensor(f"{p}_att", [1, hc], bf16,
                                  kind="ExternalInput")
        t["wla"] = nc.dram_tensor(f"{p}_wla", [65, hc], bf16,
                                  kind="ExternalInput")
        t["wra"] = nc.dram_tensor(f"{p}_wra", [65, hc], bf16,
                                  kind="ExternalInput")
        if p in ("j2", "m2"):
            t["wlb"] = nc.dram_tensor(f"{p}_wlb", [128, hc], bf16,
                                      kind="ExternalInput")
            t["wrb"] = nc.dram_tensor(f"{p}_wrb", [128, hc], bf16,
                                      kind="ExternalInput")
            t["bias"] = nc.dram_tensor(f"{p}_bias", [1, 128], f32,
                                       kind="ExternalInput")
        else:
            t["bias"] = nc.dram_tensor(f"{p}_bias", [1, 64], f32,
                                       kind="ExternalInput")
        convs[p] = t
    outp = nc.dram_tensor("outp", [npad, 128], bf16, kind="ExternalOutput")

    qc = [0]  # gather queue rotator

    with tile.TileContext(nc) as tc:
        with (
            tc.tile_pool(name="res", bufs=1) as rp,
            tc.tile_pool(name="stream", bufs=1) as sp,
            tc.tile_pool(name="gat", bufs=3) as gp,
            tc.tile_pool(name="wrk", bufs=2) as wp,
            tc.tile_pool(name="epi", bufs=1) as ep,
            tc.tile_pool(name="tb", bufs=2) as tbp,
            tc.tile_pool(name="pacc", bufs=4, space="PSUM") as pacc,
            tc.tile_pool(name="ptb", bufs=2, space="PSUM") as ptb,
            tc.tile_pool(name="dram", bufs=1, space="DRAM") as dp,
        ):
            nc.gpsimd.load_library(mlp)
            xbt_t = rp.tile([65, npad], bf16)
            nc.sync.dma_start(xbt_t[:], xbt[:])
            iota_t = rp.tile([128, 128], bf16)
            nc.sync.dma_start(iota_t[:], iotin[:].to_broadcast((128, 128)))
            ident_t = rp.tile([128, 128], bf16)
            nc.sync.dma_start(ident_t[:], identin[:])
            h1T = rp.tile([128, npad], bf16)
            stash = rp.tile([128, nb, 128], bf16)

            nsmax = max(scheds[p][1] + scheds[p][2]
                        for p in ("j1", "m1", "j2", "m2"))

            def load_streams(p, slot):
                sched, ns_lo, ns_hi = scheds[p]
                t = convs[p]
                ns = ns_lo + ns_hi
                s16_t = sp.tile([128, nsmax * 8], i16, tag=f"s16_{slot}")
                nc.sync.dma_start(s16_t[:, 0:ns * 8], t["src16"][:])
                d16_t = sp.tile([128, nsmax * 8], i16, tag=f"d16_{slot}")
                nc.sync.dma_start(d16_t[:, 0:ns * 8], t["dst16"][:])
                dloc_t = sp.tile([128, nsmax], bf16, tag=f"dloc_{slot}")
                nc.sync.dma_start(dloc_t[:, 0:ns], t["dloc"][:])
                return s16_t, d16_t, dloc_t

            def build_tables(p, hc, with_h1):
                """Own-shard XL/XR tables; AllGather XL. -> (xl_full, xr)."""
                t = convs[p]
                wla_t = rp.tile([65, hc], bf16, tag=f"w_{p}l")
                nc.sync.dma_start(wla_t[:], t["wla"][:])
                wra_t = rp.tile([65, hc], bf16, tag=f"w_{p}r")
                nc.sync.dma_start(wra_t[:], t["wra"][:])
                if with_h1:
                    wlb_t = rp.tile([128, hc], bf16, tag=f"w_{p}lb")
                    nc.sync.dma_start(wlb_t[:], t["wlb"][:])
                    wrb_t = rp.tile([128, hc], bf16, tag=f"w_{p}rb")
                    nc.sync.dma_start(wrb_t[:], t["wrb"][:])
                xl_sh = dp.tile([npad, hc], bf16, tag=f"xlsh_{p}")
                xl_full = dp.tile([npadg, hc], bf16, tag=f"xlf_{p}",
                                  addr_space="Shared")
                xr = dp.tile([npad, hc], bf16, tag=f"xr_{p}")
                for side in (0, 1):
                    wa = wla_t if side == 0 else wra_t
                    dst_d = xl_sh if side == 0 else xr
                    for jt in range(nb):
                        ps = ptb.tile([128, hc], f32, space="PSUM", tag="tb")
                        nc.tensor.matmul(ps[:], lhsT=xbt_t[:, jt * 128:(jt + 1) * 128],
                                         rhs=wa[:], start=True, stop=not with_h1)
                        if with_h1:
                            wb = wlb_t if side == 0 else wrb_t
                            nc.tensor.matmul(ps[:],
                                             lhsT=h1T[:, jt * 128:(jt + 1) * 128],
                                             rhs=wb[:], start=False, stop=True)
                        sb = tbp.tile([128, hc], bf16, tag="tbsb")
                        nc.vector.tensor_copy(sb[:], ps[:])
                        nc.sync.dma_start(dst_d[jt * 128:(jt + 1) * 128, :], sb[:])
                    if side == 0:
                        nc.gpsimd.collective_compute(
                            "AllGather", AT.bypass,
                            replica_groups=[list(range(ncores))],
                            ins=[xl_sh.opt()], outs=[xl_full.opt()])
                return xl_full, xr

            def conv_pass(p, hc, streams, epi):
                sched, ns_lo, ns_hi = scheds[p]
                t = convs[p]
                xl_full, xr = t["xlf"], t["xr"]
                s16_t, d16_t, dloc_t = streams
                att_t = rp.tile([128, hc], bf16, tag=f"att_{p}")
                nc.sync.dma_start(att_t[:], t["att"][:].to_broadcast((128, hc)))

                xl_lo = xl_full[0:split, :]
                xl_hi = xl_full[split:npadg, :]
                half_off = {"lo": 0, "hi": ns_lo}
                # chunk ordinals within each half (chunks span blocks)
                cmap, cdef = {}, {}
                for half, n in (("lo", ns_lo), ("hi", ns_hi)):
                    for cid, s0 in enumerate(range(0, n, CH)):
                        nk = min(CH, n - s0)
                        cdef[(half, cid)] = (s0, nk)
                        for j in range(nk):
                            cmap[(half, s0 + j)] = (cid, j)
                chunks = {}

                def get_chunk(half, k):
                    key = (half, k)
                    if key in chunks:
                        return chunks[key]
                    o0, nk = cdef[key]
                    g0 = half_off[half] + o0
                    nidx = nk * 128
                    gt = gp.tile([128, nk, hc], bf16, tag=f"gsrc_{half}", bufs=4)
                    nc.gpsimd.dma_gather(
                        gt[:], xl_lo if half == "lo" else xl_hi,
                        s16_t[:, g0 * 8:(g0 + nk) * 8], nidx, nidx, hc,
                        queue_num=qc[0] % 4)
                    qc[0] += 1
                    rt = gp.tile([128, nk, hc], bf16, tag=f"gdst_{half}", bufs=2)
                    nc.gpsimd.dma_gather(
                        rt[:], xr[:], d16_t[:, g0 * 8:(g0 + nk) * 8],
                        nidx, nidx, hc, queue_num=qc[0] % 4)
                    qc[0] += 1
                    et = wp.tile([128, nk, hc], bf16, tag="e")
                    nc.vector.tensor_tensor(out=et[:], in0=gt[:], in1=rt[:],
                                            op=AT.add)
                    nc.scalar.activation(et[:], et[:], AF.Prelu, alpha=0.15)
                    nc.vector.tensor_tensor(
                        out=et[:], in0=et[:],
                        in1=att_t[:].rearrange("p (a c) -> p a c", a=1)
                        .to_broadcast((128, nk, hc)), op=AT.mult)
                    lg = wp.tile([128, nk * 2], f32, tag="lg")
                    nc.vector.tensor_reduce(
                        out=lg[:].rearrange("p (a h) -> p a h", h=2),
                        in_=et[:].rearrange("p a (c h) -> p a h c", h=2),
                        axis=mybir.AxisListType.X, op=AT.add)
                    wt = wp.tile([128, nk, 2], bf16, tag=f"w_{half}")
                    nc.scalar.activation(
                        wt[:], lg[:].rearrange("p (a h) -> p a h", h=2), AF.Exp)
                    nc.vector.tensor_tensor(
                        out=gt[:].rearrange("p a (c h) -> p a c h", h=2),
                        in0=gt[:].rearrange("p a (c h) -> p a c h", h=2),
                        in1=wt[:].rearrange("p a (o h) -> p a o h", o=1)
                        .to_broadcast((128, nk, hc // 2, 2)), op=AT.mult)
                    oh = wp.tile([128, nk, 128], bf16, tag=f"oh_{half}")
                    nc.vector.tensor_tensor(
                        out=oh[:],
                        in0=dloc_t[:, g0:g0 + nk]
                        .rearrange("p (a b) -> p a b", b=1)
                        .to_broadcast((128, nk, 128)),
                        in1=iota_t[:].rearrange("p (a c) -> p a c", a=1)
                        .to_broadcast((128, nk, 128)),
                        op=AT.is_equal)
                    ck = {"gt": gt, "wt": wt, "oh": oh}
                    chunks[key] = ck
                    return ck

                stg = ep.tile([128, nb, hc + 2], bf16,
                              tag="stg1" if hc == D1 else "stg2")
                olo, ohi = 0, 0
                for b in range(nb):
                    nlo, nhi = sched[b]
                    subs = ([("lo", olo + i) for i in range(nlo)]
                            + [("hi", ohi + i) for i in range(nhi)])
                    olo += nlo
                    ohi += nhi
                    acc = pacc.tile([128, hc + 2], f32, space="PSUM", tag="acc")
                    n = len(subs)
                    for si, (half, o) in enumerate(subs):
                        cid, s = cmap[(half, o)]
                        ck = get_chunk(half, cid)
                        nc.tensor.matmul(acc[:, 0:hc], lhsT=ck["oh"][:, s, :],
                                         rhs=ck["gt"][:, s, :],
                                         start=(si == 0), stop=(si == n - 1))
                    for si, (half, o) in enumerate(subs):
                        cid, s = cmap[(half, o)]
                        ck = get_chunk(half, cid)
                        nc.tensor.matmul(acc[:, hc:hc + 2], lhsT=ck["oh"][:, s, :],
                                         rhs=ck["wt"][:, s, :],
                                         start=(si == 0), stop=(si == n - 1))
                    nc.scalar.activation(stg[:, b, :], acc[:], AF.Copy)
                epi(stg)

            def batched_norm(p, hc, stg):
                """stg [128, nb, hc+2] bf16 -> st [128, nb, hc/2] bf16
                (normalized, head-mean'd via 0.5*recipden, + bias)."""
                bw = hc // 2
                L = "1" if hc == D1 else "2"
                bias_t = rp.tile([128, bw], f32, tag=f"bias_{p}")
                nc.sync.dma_start(bias_t[:],
                                  convs[p]["bias"][:].to_broadcast((128, bw)))
                dn = ep.tile([128, nb, 2], f32, tag=f"dn{L}")
                nc.vector.tensor_scalar(out=dn[:], in0=stg[:, :, hc:hc + 2],
                                        scalar1=1e-16, scalar2=None, op0=AT.add)
                nc.vector.reciprocal(dn[:], dn[:])
                nc.vector.tensor_scalar(out=dn[:], in0=dn[:], scalar1=0.5,
                                        scalar2=None, op0=AT.mult)
                vals = stg[:, :, 0:hc].rearrange("p b (c h) -> p b c h", h=2)
                nc.vector.tensor_tensor(
                    out=vals, in0=vals,
                    in1=dn[:].rearrange("p b (o h) -> p b o h", o=1)
                    .to_broadcast((128, nb, bw, 2)), op=AT.mult)
                st = ep.tile([128, nb, bw], bf16, tag=f"st{L}")
                nc.vector.tensor_tensor(out=st[:], in0=stg[:, :, 0:hc]
                                        .rearrange("p b (c h) -> p b c h", h=2)[:, :, :, 0],
                                        in1=stg[:, :, 0:hc]
                                        .rearrange("p b (c h) -> p b c h", h=2)[:, :, :, 1],
                                        op=AT.add)
                nc.vector.tensor_tensor(
                    out=st[:], in0=st[:],
                    in1=bias_t[:].rearrange("p (a c) -> p a c", a=1)
                    .to_broadcast((128, nb, bw)), op=AT.add)
                return st

            def epi_l1(p, rowoff):
                def f(stg):
                    st = batched_norm(p, D1, stg)
                    nc.scalar.activation(st[:], st[:], AF.Relu)
                    for b in range(nb):
                        tp = ptb.tile([64, 128], bf16, space="PSUM", tag="tr")
                        nc.tensor.transpose(tp[:], st[:, b, :], ident_t[:])
                        nc.vector.tensor_copy(
                            h1T[rowoff:rowoff + 64, b * 128:(b + 1) * 128], tp[:])
                return f

            def epi_l2(p, is_j):
                def f(stg):
                    st = batched_norm(p, D2, stg)
                    if is_j:
                        nc.vector.tensor_copy(stash[:], st[:])
                    else:
                        nc.vector.tensor_tensor(out=st[:], in0=st[:],
                                                in1=stash[:], op=AT.add)
                        nc.scalar.activation(st[:], st[:], AF.Relu, scale=0.5)
                        nc.sync.dma_start(
                            outp[:].rearrange("(b q) c -> q b c", b=nb), st[:])
                return f

            # ---- layer 1 ----
            str_j1 = load_streams("j1", "A")
            str_m1 = load_streams("m1", "B")
            for p in ("j1", "m1"):
                fl, r = build_tables(p, D1, False)
                convs[p]["xlf"], convs[p]["xr"] = fl, r
            conv_pass("j1", D1, str_j1, epi_l1("j1", 0))
            str_j2 = load_streams("j2", "A")
            conv_pass("m1", D1, str_m1, epi_l1("m1", 64))
            str_m2 = load_streams("m2", "B")

            # ---- layer 2 ----
            for p in ("j2", "m2"):
                fl, r = build_tables(p, D2, True)
                convs[p]["xlf"], convs[p]["xr"] = fl, r
            conv_pass("j2", D2, str_j2, epi_l2("j2", True))
            conv_pass("m2", D2, str_m2, epi_l2("m2", False))

    nc.compile()
    return nc


def run_full(inputs, N, ncores, split=32768, npc=None):
    """Host orchestration: preprocess, build, run, assemble output."""
    x = np.asarray(inputs["x"], np.float32)
    npc = npc or N // ncores
    npad = ((npc + 127) // 128) * 128
    IN = x.shape[1]

    def prep_edges(e):
        e = np.asarray(e)
        return e[0].astype(np.int64), e[1].astype(np.int64)

    js, jd = prep_edges(inputs["job_edges"])
    ms, md = prep_edges(inputs["mac_edges"])

    scheds, streams = {}, {}
    for p, (s, d) in [("j1", (js, jd)), ("m1", (ms, md)),
                      ("j2", (js, jd)), ("m2", (ms, md))]:
        sch, nlo, nhi, pc = preprocess_conv(s, d, ncores, npc, npad, split)
        scheds[p] = (sch, nlo, nhi)
        streams[p] = pc

    nc = build_program(ncores, npc, npad, split, scheds)

    xall = np.zeros((65, ncores * npad), np.float32)
    for c in range(ncores):
        n0 = c * npc
        w = min(npc, N - n0) if n0 < N else 0
        if w > 0:
            xall[0:IN, c * npad:c * npad + w] = x[n0:n0 + w].T
    xall[64, :] = 1.0
    xall = xall.astype(ml_dtypes.bfloat16)

    iot = np.arange(128, dtype=np.float32)[None, :].astype(ml_dtypes.bfloat16)
    ident = np.eye(128, dtype=np.float32).astype(ml_dtypes.bfloat16)

    consts = {}
    for p, key in [("j1", "jg1"), ("m1", "mg1"), ("j2", "jg2"), ("m2", "mg2")]:
        Wl = np.asarray(inputs[key + "_Wl"], np.float32)
        bl = np.asarray(inputs[key + "_bl"], np.float32)
        Wr = np.asarray(inputs[key + "_Wr"], np.float32)
        br = np.asarray(inputs[key + "_br"], np.float32)
        att = np.asarray(inputs[key + "_att"], np.float32)
        b = np.asarray(inputs[key + "_b"], np.float32)
        hc = Wl.shape[1]
        C = hc // 2
        # head-minor column order: new col (c, h) = c*2+h <- old h*C+c
        perm = (np.arange(hc).reshape(C, 2).T.flatten()
                .reshape(2, C))  # perm[h, c] = old index h*C+c... build directly:
        perm = np.array([[h * C + c for h in range(2)] for c in range(C)]
                        ).flatten()
        Wl = Wl[:, perm]
        Wr = Wr[:, perm]
        bl = bl[perm]
        br = br[perm]
        attf = att.flatten()[perm]
        d = {}
        d[f"{p}_att"] = attf.reshape(1, hc).astype(ml_dtypes.bfloat16)
        d[f"{p}_bias"] = b[None, :].astype(np.float32)
        if p in ("j1", "m1"):
            d[f"{p}_wla"] = aug_w(Wl, bl)
            d[f"{p}_wra"] = aug_w(Wr, br)
        else:
            d[f"{p}_wla"] = aug_w(Wl[0:64], bl)
            d[f"{p}_wlb"] = Wl[64:192].astype(ml_dtypes.bfloat16)
            d[f"{p}_wra"] = aug_w(Wr[0:64], br)
            d[f"{p}_wrb"] = Wr[64:192].astype(ml_dtypes.bfloat16)
        consts.update(d)

    in_maps = []
    for c in range(ncores):
        m = {"xbt": xall[:, c * npad:(c + 1) * npad].copy(),
             "iot": iot, "ident": ident}
        m.update(consts)
        for p in ("j1", "m1", "j2", "m2"):
            st = streams[p][c]
            m[f"{p}_src16"] = st["src16"]
            m[f"{p}_dst16"] = st["dst16"]
            m[f"{p}_dloc"] = np.ascontiguousarray(st["dloc"])
        in_maps.append(m)

    res = run_bass_kernel_spmd(nc, in_maps, list(range(ncores)),
                               trace=bool(int(os.environ.get('GAT_TRACE', '0'))))
    parts = [np.asarray(res.results[c]["outp"][:min(npc, N - c * npc)],
                        dtype=np.float32) for c in range(ncores)]
    out = np.concatenate([x, np.vstack(parts)], axis=1).astype(np.float32)
    return out, res


def _install_profile_shim():
    """Optional: register the NTFF profiling hook (GAT_TRACE=1)."""
    try:
        import types
        import antenv
        if "antenv.axon_hooks" not in sys.modules:
            _store = {}
            m = types.ModuleType("antenv.axon_hooks")
            m.set_axon_ntff_profile_hook = lambda h: _store.__setitem__("h", h)
            m.get_axon_ntff_profile_hook = lambda: _store.get("h")
            sys.modules["antenv.axon_hooks"] = m
            antenv.axon_hooks = m
        from trn_agent_boot.trn_boot import _ntff_profile_via_ctypes
        sys.modules["antenv.axon_hooks"].set_axon_ntff_profile_hook(
            _ntff_profile_via_ctypes("/opt/axon/libaxon_pjrt.so"))
    except Exception:
        pass


LAST_RESULT = None


def kernel(**inputs):
    global LAST_RESULT
    if os.environ.get("GAT_TRACE", "0") == "1":
        _install_profile_shim()
    out, res = run_full(inputs, 50000, 8)
    LAST_RESULT = res
    return out
